# revision 17
# baseline (speedup 1.0000x reference)
"""Trainium2 Bass kernel for nn_CNNInteractLayer (CNN interaction layer).

Math: for each episode b, s-row i, q-row j:
  out[b,i,j] = maxpool_L(relu(conv_k(concat(s[b,i], q[b,j])))) for k in 2..5
Factorization: conv(concat(s,q)) = conv_s(s) + conv_q(q) + bias, so per-row
convolutions run once on the PE (bf16, fp32 psum), and the pairwise stage is
fused on the vector engine: the scalar engine evicts conv psum to SBUF bf16
(s-side with bias folded in), then the DVE forms pairwise sums with a
broadcast tensor_tensor add (2x bf16 mode) and runs a 5-level binary max
tree (tensor_tensor max at 2x beats tensor_reduce/pool at 1x). No pairwise
matmul, no A-matrix, no DRAM transpose roundtrip.

Sharding: 8 cores = 4 episodes x 2 halves of the q-row range.
"""

import os
import sys

import numpy as np

for _p in ("/opt/trn_rl_repo",):
    if os.path.isdir(_p) and _p not in sys.path:
        sys.path.insert(0, _p)

# the bass runner needs the axon jax backend; don't let a cpu-only pin hide it
if "axon" not in os.environ.get("JAX_PLATFORMS", "axon"):
    os.environ.pop("JAX_PLATFORMS", None)

import ml_dtypes  # noqa: E402

from concourse import bacc, bass, mybir, tile  # noqa: E402
from concourse.bass_utils import run_bass_kernel_spmd  # noqa: E402

BF16 = np.dtype(ml_dtypes.bfloat16)

# Problem dims (hardcoded per spec)
B, N, K, Q, L, D = 4, 5, 5, 5, 31, 512
NROW = N * K            # 25 s-rows per episode
NQROW = N * Q           # 25 q-rows per episode
JN = 13                 # q-rows per core (padded; odd cores use 12)
ROWSTR = L + 2          # 33: 2-zero gap between rows gives conv zero-padding
PS_COLS = NROW * ROWSTR + 9   # 834 (row r data at r*33+4 .. +34; halo right)
PQ_COLS = JN * ROWSTR + 9     # 438
S_OUT = 826             # conv output positions computed, s side (even)
Q_OUT = 430             # q side (even)
SLAB_S = 828            # per-chunk slab stride in cs_sb
SLAB_Q = 432
NCH = 600               # device channels: [k5 | k4 | k3 | k2] x 150
# delta (tap shift) groups; prefix-size in device channel order
DELTAS = [(-2, 300), (-1, 600), (0, 600), (1, 450), (2, 150)]
# emission order per d-chunk: full-coverage groups first so the first matmul
# of each PSUM accumulation group writes the full partition range
DORDER = [1, 2, 0, 3, 4]
WOFF = [0, 300, 900, 1500, 1950]  # packed col offset of each delta group
WSIDE = 2100
CC0 = [0, 128, 256, 384, 512]     # channel chunk starts
CCW = [128, 128, 128, 128, 88]
NPAIR = NROW * JN                 # 325
PAD_OF_K = {2: 1, 3: 1, 4: 2, 5: 2}
ORD_OF_K = {5: 0, 4: 1, 3: 2, 2: 3}
POSCH_S = [(0, 430), (430, 396)]  # row-aligned: rows 0-12 | 13-24
POSCH_Q = [(0, 430)]
# conv emission order: smallest chunks first to fill the DVE/Pool pipe early
CCORDER = [4, 3, 2, 1, 0]
JBLOCKS = [(0, 13)]  # pairwise j-blocks (single: fewer DVE instr inits)

# chunk-major packed-W layout: per channel chunk, [side s | side q], each a
# concatenation of the valid delta groups' column slices for that chunk
def _chunk_tables():
    chw = []          # per-side width of each chunk block
    coloff = {}       # (cc, side, di) -> column offset in packed W
    off = 0
    for cc in range(5):
        c0 = CC0[cc]
        widths = []
        for di, (_, sz) in enumerate(DELTAS):
            w = min(128, sz - c0) if sz > c0 else 0
            widths.append(w)
        side_w = sum(widths)
        for side in range(2):
            p = off + side * side_w
            for di, w in enumerate(widths):
                if w:
                    coloff[(cc, side, di)] = p
                    p += w
        chw.append(side_w)
        off += 2 * side_w
    return chw, coloff


CHW, WCOL = _chunk_tables()
CHOFF = [sum(2 * w for w in CHW[:i]) for i in range(6)]

_PROG = None


def _build_program():
    nc = bacc.Bacc("TRN2", target_bir_lowering=False, debug=False, num_devices=8)
    f32 = mybir.dt.float32
    bf16 = mybir.dt.bfloat16

    ps_d = nc.dram_tensor("ps", [D, PS_COLS], bf16, kind="ExternalInput")
    pq_d = nc.dram_tensor("pq", [D, PQ_COLS], bf16, kind="ExternalInput")
    w_d = nc.dram_tensor("w", [D, 2 * WSIDE], bf16, kind="ExternalInput")
    bias_d = nc.dram_tensor("bias", [640, 1], f32, kind="ExternalInput")
    out_d = nc.dram_tensor("out", [NCH, NPAIR], bf16, kind="ExternalOutput")

    with tile.TileContext(nc) as tc:
        with (
            tc.tile_pool(name="persist", bufs=1) as big,
            tc.tile_pool(name="tmppool", bufs=2) as tmppool,
            tc.tile_pool(name="t16pool", bufs=2) as t16pool,
            tc.tile_pool(name="t8pool", bufs=2) as t8pool,
            tc.tile_pool(name="t4pool", bufs=2) as t4pool,
            tc.tile_pool(name="t2pool", bufs=2) as t2pool,
            tc.tile_pool(name="convps", bufs=3, space="PSUM") as convps,
        ):
            w_sb = big.tile([128, 4 * 2 * WSIDE], bf16, tag="w")
            ps_sb = big.tile([128, 4 * PS_COLS], bf16, tag="ps")
            pq_sb = big.tile([128, 4 * PQ_COLS], bf16, tag="pq")
            cs_sb = big.tile([128, 5 * SLAB_S], bf16, tag="cs")
            cq_sb = big.tile([128, 5 * SLAB_Q], bf16, tag="cq")
            bias_sb = big.tile([128, 6], f32, tag="bias")
            red = big.tile([128, 5 * NPAIR], bf16, tag="red")

            def wload(cc, eng=None):
                wd = w_d[:].rearrange("(d p) c -> p d c", p=128)
                ws = w_sb[:].rearrange("p (d c) -> p d c", c=2 * WSIDE)
                (eng or nc.sync).dma_start(
                    ws[:, :, CHOFF[cc] : CHOFF[cc + 1]],
                    wd[:, :, CHOFF[cc] : CHOFF[cc + 1]],
                )

            # two parallel HWDGE queues; pieces ordered so the first
            # chunk's conv inputs land first: SP interleaves w-chunk-d with
            # pq-d (q-conv consumption order), Act carries bias then ps in
            # poschunk-A/B column halves per d
            wd3 = w_d[:].rearrange("(d p) c -> p d c", p=128)
            ws3 = w_sb[:].rearrange("p (d c) -> p d c", c=2 * WSIDE)
            pq3 = pq_sb[:].rearrange("p (d c) -> p d c", c=PQ_COLS)
            qd3 = pq_d[:].rearrange("(d p) c -> p d c", p=128)
            c4a, c4b = CHOFF[CCORDER[0]], CHOFF[CCORDER[0] + 1]
            for d in range(4):
                nc.sync.dma_start(
                    ws3[:, d : d + 1, c4a:c4b], wd3[:, d : d + 1, c4a:c4b]
                )
                nc.sync.dma_start(pq3[:, d : d + 1, :], qd3[:, d : d + 1, :])
            # bias[c] at dram row c -> bias_sb[p, cc] for c = cc*128+p
            # (first on the Act queue: it gates the first s-side evict)
            nc.scalar.dma_start(
                bias_sb[:, 0:5],
                bass.AP(bias_d[:].tensor, 0, [[1, 128], [128, 5]]),
            )
            ps3 = ps_sb[:].rearrange("p (d c) -> p d c", c=PS_COLS)
            pd3 = ps_d[:].rearrange("(d p) c -> p d c", p=128)
            for lo, hi in ((0, 434), (434, PS_COLS)):
                for d in range(4):
                    nc.scalar.dma_start(
                        ps3[:, d : d + 1, lo:hi], pd3[:, d : d + 1, lo:hi]
                    )
            nc.vector.memset(bias_sb[:, 5:6], 0.0)
            # touch the activation table during the DMA prologue so the
            # 1.3us LoadActFuncSet is off the critical path
            nc.scalar.activation(
                bias_sb[:, 5:6], bias_sb[:, 5:6],
                mybir.ActivationFunctionType.Relu,
            )
            wload(CCORDER[1])

            def conv(cc, side, poschunks=None):
                """Conv for channel chunk cc of one side -> cs/cq slab."""
                c0, ccw = CC0[cc], CCW[cc]
                src, dst, pdef, cols, slab, bcol = (
                    (ps_sb, cs_sb, POSCH_S, PS_COLS, SLAB_S, cc)
                    if side == 0
                    else (pq_sb, cq_sb, POSCH_Q, PQ_COLS, SLAB_Q, 5)
                )
                poschunks = poschunks or pdef
                src3 = src[:].rearrange("p (d c) -> p d c", c=cols)
                for pos0, pw in poschunks:
                    psum = convps.tile([128, 432], f32, tag="conv")
                    mms = []
                    for d in range(4):
                        # first and last matmul of the accumulation group must
                        # cover the full partition range (start/stop semantics
                        # are per-element), so full-size delta groups bracket
                        order = DORDER if d < 3 else [1, 0, 3, 4, 2]
                        for di in order:
                            delta, sz = DELTAS[di]
                            if sz <= c0:
                                continue
                            wcc = min(ccw, sz - c0)
                            mms.append((d, di, delta, wcc))
                    for idx, (d, di, delta, wcc) in enumerate(mms):
                        lcol = d * 2 * WSIDE + WCOL[(cc, side, di)]
                        rcol = pos0 + delta + 2
                        nc.tensor.matmul(
                            psum[0:wcc, 0:pw],
                            lhsT=w_sb[:, lcol : lcol + wcc],
                            rhs=src3[:, d, rcol : rcol + pw],
                            start=(idx == 0),
                            stop=(idx == len(mms) - 1),
                        )
                    # psum -> SBUF bf16 on the scalar engine; bias folds into
                    # the s side so each pairwise sum gets it exactly once
                    nc.scalar.add(
                        dst[0:ccw, cc * slab + pos0 : cc * slab + pos0 + pw],
                        psum[0:ccw, 0:pw],
                        bias_sb[0:ccw, bcol : bcol + 1],
                    )

            def pairwise(cc, last=False, first=False):
                """Fused pairwise add + maxpool for chunk cc.

                tmp[ch,j,i,l] = cs[ch,i,l] + cq[ch,j,l]   (DVE, 2x bf16)
                L1 16-wide max on DVE, L2..L5 on GPSIMD, relu on DVE.
                """
                ccw = CCW[cc]
                mx = mybir.AluOpType.max
                for j0, jb in JBLOCKS:
                    npr = jb * NROW
                    tmp = tmppool.tile([128, JN * NROW * 31], bf16, tag="tmp")
                    t16 = t16pool.tile([128, JN * NROW * 16], bf16, tag="t16")
                    t8 = t8pool.tile([128, JN * NROW * 8], bf16, tag="t8")
                    t4 = t4pool.tile([128, JN * NROW * 4], bf16, tag="t4")
                    t2 = t2pool.tile([128, JN * NROW * 2], bf16, tag="t2")
                    def ap(t, off, dims):
                        tap = t[:]
                        return bass.AP(
                            tap.tensor,
                            tap.offset + off,
                            [[tap.ap[0][0], ccw]] + dims,
                        )

                    # first chunk: quadrant split so each add piece is
                    # gated on partial conv evicts (pipeline fill); elsewhere
                    # one instruction saves init overhead
                    pieces = (
                        ((0, 7, 0, 13), (0, 7, 13, 12),
                         (7, 6, 0, 13), (7, 6, 13, 12))
                        if first else ((0, jb, 0, 25),)
                    )
                    for jp0, nj, i0, ni in pieces:
                        nc.vector.tensor_tensor(
                            ap(tmp, jp0 * 775 + i0 * 31,
                               [[775, nj], [31, ni], [1, 31]]),
                            ap(cs_sb, cc * SLAB_S + 2 + i0 * ROWSTR,
                               [[0, nj], [ROWSTR, ni], [1, 31]]),
                            ap(cq_sb,
                               cc * SLAB_Q + 2 + (j0 + jp0) * ROWSTR,
                               [[ROWSTR, nj], [0, ni], [1, 31]]),
                            op=mybir.AluOpType.add,
                        )
                    nc.vector.tensor_tensor(
                        ap(t16, 0, [[16, npr], [1, 16]]),
                        ap(tmp, 0, [[31, npr], [1, 16]]),
                        ap(tmp, 15, [[31, npr], [1, 16]]),
                        op=mx,
                    )
                    # tree levels L2..L5 + relu + store; for the last
                    # chunk run them in pair-range pieces so the final out
                    # DMA overlaps the tail of the tree
                    pieces = ((0, 225), (225, 100)) if last else ((0, npr),)
                    for p0, pn in pieces:
                        nc.vector.tensor_tensor(
                            ap(t8, p0 * 8, [[8, pn], [1, 8]]),
                            ap(t16, p0 * 16, [[16, pn], [1, 8]]),
                            ap(t16, p0 * 16 + 8, [[16, pn], [1, 8]]),
                            op=mx,
                        )
                        nc.vector.tensor_tensor(
                            ap(t4, p0 * 4, [[4, pn], [1, 4]]),
                            ap(t8, p0 * 8, [[8, pn], [1, 4]]),
                            ap(t8, p0 * 8 + 4, [[8, pn], [1, 4]]),
                            op=mx,
                        )
                        nc.vector.tensor_tensor(
                            ap(t2, p0 * 2, [[2, pn], [1, 2]]),
                            ap(t4, p0 * 4, [[4, pn], [1, 2]]),
                            ap(t4, p0 * 4 + 2, [[4, pn], [1, 2]]),
                            op=mx,
                        )
                        nc.vector.tensor_tensor(
                            ap(red, cc * NPAIR + j0 * NROW + p0, [[1, pn]]),
                            ap(t2, p0 * 2, [[2, pn]]),
                            ap(t2, p0 * 2 + 1, [[2, pn]]),
                            op=mx,
                        )
                        if last:
                            nc.vector.tensor_scalar_max(
                                red[0:ccw, cc * NPAIR + p0 : cc * NPAIR + p0 + pn],
                                red[0:ccw, cc * NPAIR + p0 : cc * NPAIR + p0 + pn],
                                0.0,
                            )
                            nc.sync.dma_start(
                                out_d[CC0[cc] : CC0[cc] + ccw, p0 : p0 + pn],
                                red[0:ccw, cc * NPAIR + p0 : cc * NPAIR + p0 + pn],
                            )
                if not last:
                    nc.scalar.activation(
                        red[0:ccw, cc * NPAIR : (cc + 1) * NPAIR],
                        red[0:ccw, cc * NPAIR : (cc + 1) * NPAIR],
                        mybir.ActivationFunctionType.Relu,
                    )
                    nc.sync.dma_start(
                        out_d[CC0[cc] : CC0[cc] + ccw, :],
                        red[0:ccw, cc * NPAIR : (cc + 1) * NPAIR],
                    )

            # software pipeline: conv leads pairwise by one chunk
            conv(CCORDER[0], 1, poschunks=[(0, 232), (232, 198)])
            conv(CCORDER[0], 0)
            for k in range(1, 5):
                if k + 1 <= 4:
                    wload(CCORDER[k + 1])
                conv(CCORDER[k], 1)
                conv(CCORDER[k], 0)
                pairwise(CCORDER[k - 1], first=(k == 1))
            pairwise(CCORDER[4], last=True)

    nc.compile()
    return nc


def get_program():
    global _PROG
    if _PROG is None:
        _PROG = _build_program()
    return _PROG


def build_inputs(s, q, ws, bs):
    """Host-side shard prep. ws/bs: dicts k -> w(150, 1024, k) / b(150,).

    Returns in_maps. Core c handles episode c//2, q-row half c%2.
    """
    s = np.asarray(s, dtype=np.float32).reshape(B, NROW, L, D)
    q = np.asarray(q, dtype=np.float32).reshape(B, NQROW, L, D)

    # packed weights [D, 2*2100]: per side, delta groups at WOFF offsets,
    # device channel order [k5|k4|k3|k2]
    wall = np.zeros((D, 2 * WSIDE), dtype=np.float32)
    bias_dev = np.zeros(640, dtype=np.float32)
    for k in (2, 3, 4, 5):
        blk = ORD_OF_K[k] * 150
        bias_dev[blk : blk + 150] = bs[k]
        for di, (delta, sz) in enumerate(DELTAS):
            t = delta + PAD_OF_K[k]
            if not (0 <= t < k):
                continue
            assert blk + 150 <= sz
            wall[:, WOFF[di] + blk : WOFF[di] + blk + 150] = ws[k][:, :D, t].T
            wall[:, WSIDE + WOFF[di] + blk : WSIDE + WOFF[di] + blk + 150] = (
                ws[k][:, D:, t].T
            )
    perm = np.zeros(2 * WSIDE, dtype=np.int64)
    for side in range(2):
        for di, (_, sz) in enumerate(DELTAS):
            for cc in range(5):
                c0 = CC0[cc]
                if sz <= c0:
                    continue
                w = min(128, sz - c0)
                newc = WCOL[(cc, side, di)]
                oldc = side * WSIDE + WOFF[di] + c0
                perm[newc : newc + w] = np.arange(oldc, oldc + w)
    wall = wall[:, perm].astype(BF16)
    bias_col = bias_dev[:, None]

    in_maps = []
    for core in range(8):
        b, jh = core // 2, core % 2
        jidx = [min(jh * JN + t, NQROW - 1) for t in range(JN)]
        psa = np.zeros((D, PS_COLS), dtype=np.float32)
        pqa = np.zeros((D, PQ_COLS), dtype=np.float32)
        for r in range(NROW):
            psa[:, r * ROWSTR + 4 : r * ROWSTR + 4 + L] = s[b, r].T
        for t, j in enumerate(jidx):
            pqa[:, t * ROWSTR + 4 : t * ROWSTR + 4 + L] = q[b, j].T
        in_maps.append(
            {
                "ps": psa.astype(BF16),
                "pq": pqa.astype(BF16),
                "w": wall,
                "bias": bias_col,
            }
        )
    return in_maps


# device channel -> original output channel maps
_S_IDX = np.array(
    [(3 - g) * 150 + u for g in range(4) for u in range(75)], dtype=np.int64
)
_Q_IDX = _S_IDX + 75


def assemble_outputs(core_outs):
    """core_outs: list of 8 arrays [NCH, NPAIR] -> (s_out, q_out)."""
    s_out = np.empty((B, NROW, NQROW, 300), dtype=np.float32)
    q_out = np.empty((B, NROW, NQROW, 300), dtype=np.float32)
    for core in range(8):
        b, jh = core // 2, core % 2
        nj = JN if jh == 0 else NQROW - JN
        # out[ch, j*25+i] -> [j, i, ch]
        arr = (
            np.asarray(core_outs[core])
            .astype(np.float32)
            .reshape(NCH, JN, NROW)
            .transpose(1, 2, 0)
        )
        s_out[b, :, jh * JN : jh * JN + nj] = arr[:nj][:, :, _S_IDX].transpose(
            1, 0, 2
        )
        q_out[b, :, jh * JN : jh * JN + nj] = arr[:nj][:, :, _Q_IDX].transpose(
            1, 0, 2
        )
    return s_out.reshape(-1, 300), q_out.reshape(-1, 300)


def kernel(s, q, w2, b2, w3, b3, w4, b4, w5, b5, B=4, N=5, K=5, Q=5, L=31):
    ws = {2: np.asarray(w2, np.float32), 3: np.asarray(w3, np.float32),
          4: np.asarray(w4, np.float32), 5: np.asarray(w5, np.float32)}
    bs = {2: np.asarray(b2, np.float32), 3: np.asarray(b3, np.float32),
          4: np.asarray(b4, np.float32), 5: np.asarray(b5, np.float32)}
    in_maps = build_inputs(s, q, ws, bs)
    nc = get_program()
    res = run_bass_kernel_spmd(nc, in_maps, list(range(8))).results
    return assemble_outputs([res[c]["out"] for c in range(8)])


# revision 18
# speedup vs baseline: 1.0540x; 1.0540x over previous
"""Trainium2 Bass kernel for nn_CNNInteractLayer (CNN interaction layer).

Math: for each episode b, s-row i, q-row j:
  out[b,i,j] = maxpool_L(relu(conv_k(concat(s[b,i], q[b,j])))) for k in 2..5
Factorization: conv(concat(s,q)) = conv_s(s) + conv_q(q) + bias, so per-row
convolutions run once on the PE (bf16, fp32 psum), and the pairwise stage is
fused on the vector engine: the scalar engine evicts conv psum to SBUF bf16
(s-side with bias folded in), then the DVE forms pairwise sums with a
broadcast tensor_tensor add (2x bf16 mode) and runs a 5-level binary max
tree (tensor_tensor max at 2x beats tensor_reduce/pool at 1x). No pairwise
matmul, no A-matrix, no DRAM transpose roundtrip.

Sharding: 8 cores = 4 episodes x 2 halves of the q-row range.
"""

import os
import sys

import numpy as np

for _p in ("/opt/trn_rl_repo",):
    if os.path.isdir(_p) and _p not in sys.path:
        sys.path.insert(0, _p)

# the bass runner needs the axon jax backend; don't let a cpu-only pin hide it
if "axon" not in os.environ.get("JAX_PLATFORMS", "axon"):
    os.environ.pop("JAX_PLATFORMS", None)

import ml_dtypes  # noqa: E402

from concourse import bacc, bass, mybir, tile  # noqa: E402
from concourse.bass_utils import run_bass_kernel_spmd  # noqa: E402

BF16 = np.dtype(ml_dtypes.bfloat16)

# Problem dims (hardcoded per spec)
B, N, K, Q, L, D = 4, 5, 5, 5, 31, 512
NROW = N * K            # 25 s-rows per episode
NQROW = N * Q           # 25 q-rows per episode
JN = 13                 # q-rows per core (padded; odd cores use 12)
ROWSTR = L + 2          # 33: 2-zero gap between rows gives conv zero-padding
PS_COLS = NROW * ROWSTR + 9   # 834 (row r data at r*33+4 .. +34; halo right)
PQ_COLS = JN * ROWSTR + 9     # 438
S_OUT = 826             # conv output positions computed, s side (even)
Q_OUT = 430             # q side (even)
SLAB_S = 828            # per-chunk slab stride in cs_sb
SLAB_Q = 432
NCH = 600               # device channels: [k5 | k4 | k3 | k2] x 150
# delta (tap shift) groups; prefix-size in device channel order
DELTAS = [(-2, 300), (-1, 600), (0, 600), (1, 450), (2, 150)]
# emission order per d-chunk: full-coverage groups first so the first matmul
# of each PSUM accumulation group writes the full partition range
DORDER = [1, 2, 0, 3, 4]
WOFF = [0, 300, 900, 1500, 1950]  # packed col offset of each delta group
WSIDE = 2100
CC0 = [0, 128, 256, 384, 512]     # channel chunk starts
CCW = [128, 128, 128, 128, 88]
NPAIR = NROW * JN                 # 325
PAD_OF_K = {2: 1, 3: 1, 4: 2, 5: 2}
ORD_OF_K = {5: 0, 4: 1, 3: 2, 2: 3}
POSCH_S = [(0, 430), (430, 396)]  # row-aligned: rows 0-12 | 13-24
POSCH_Q = [(0, 430)]
# conv emission order: smallest chunks first to fill the DVE/Pool pipe early
CCORDER = [4, 3, 2, 1, 0]
JBLOCKS = [(0, 13)]  # pairwise j-blocks (single: fewer DVE instr inits)

# chunk-major packed-W layout: per channel chunk, [side s | side q], each a
# concatenation of the valid delta groups' column slices for that chunk
def _chunk_tables():
    chw = []          # per-side width of each chunk block
    coloff = {}       # (cc, side, di) -> column offset in packed W
    off = 0
    for cc in range(5):
        c0 = CC0[cc]
        widths = []
        for di, (_, sz) in enumerate(DELTAS):
            w = min(128, sz - c0) if sz > c0 else 0
            widths.append(w)
        side_w = sum(widths)
        for side in range(2):
            p = off + side * side_w
            for di, w in enumerate(widths):
                if w:
                    coloff[(cc, side, di)] = p
                    p += w
        chw.append(side_w)
        off += 2 * side_w
    return chw, coloff


CHW, WCOL = _chunk_tables()
CHOFF = [sum(2 * w for w in CHW[:i]) for i in range(6)]

_PROG = None


def _build_program():
    nc = bacc.Bacc("TRN2", target_bir_lowering=False, debug=False, num_devices=8)
    f32 = mybir.dt.float32
    bf16 = mybir.dt.bfloat16

    ps_d = nc.dram_tensor("ps", [D, PS_COLS], bf16, kind="ExternalInput")
    pq_d = nc.dram_tensor("pq", [D, PQ_COLS], bf16, kind="ExternalInput")
    w_d = nc.dram_tensor("w", [D, 2 * WSIDE], bf16, kind="ExternalInput")
    bias_d = nc.dram_tensor("bias", [640, 1], f32, kind="ExternalInput")
    out_d = nc.dram_tensor("out", [NCH, NPAIR], bf16, kind="ExternalOutput")

    with tile.TileContext(nc) as tc:
        with (
            tc.tile_pool(name="persist", bufs=1) as big,
            tc.tile_pool(name="tmppool", bufs=2) as tmppool,
            tc.tile_pool(name="t16pool", bufs=2) as t16pool,
            tc.tile_pool(name="t8pool", bufs=2) as t8pool,
            tc.tile_pool(name="t4pool", bufs=2) as t4pool,
            tc.tile_pool(name="t2pool", bufs=2) as t2pool,
            tc.tile_pool(name="convps", bufs=3, space="PSUM") as convps,
        ):
            w_sb = big.tile([128, 4 * 2 * WSIDE], bf16, tag="w")
            ps_sb = big.tile([128, 4 * PS_COLS], bf16, tag="ps")
            pq_sb = big.tile([128, 4 * PQ_COLS], bf16, tag="pq")
            cs_sb = big.tile([128, 5 * SLAB_S], bf16, tag="cs")
            cq_sb = big.tile([128, 5 * SLAB_Q], bf16, tag="cq")
            bias_sb = big.tile([128, 6], f32, tag="bias")
            red = big.tile([128, 5 * NPAIR], bf16, tag="red")

            def wload(cc, eng=None):
                wd = w_d[:].rearrange("(d p) c -> p d c", p=128)
                ws = w_sb[:].rearrange("p (d c) -> p d c", c=2 * WSIDE)
                (eng or nc.sync).dma_start(
                    ws[:, :, CHOFF[cc] : CHOFF[cc + 1]],
                    wd[:, :, CHOFF[cc] : CHOFF[cc + 1]],
                )

            # two parallel HWDGE queues: SP carries w + pq, Act carries
            # bias (first: it gates the first s-side evict) then ps
            wload(CCORDER[0])
            pq3 = pq_sb[:].rearrange("p (d c) -> p d c", c=PQ_COLS)
            qd3 = pq_d[:].rearrange("(d p) c -> p d c", p=128)
            for d in range(4):
                nc.sync.dma_start(pq3[:, d : d + 1, :], qd3[:, d : d + 1, :])
            # bias[c] at dram row c -> bias_sb[p, cc] for c = cc*128+p
            nc.scalar.dma_start(
                bias_sb[:, 0:5],
                bass.AP(bias_d[:].tensor, 0, [[1, 128], [128, 5]]),
            )
            ps3 = ps_sb[:].rearrange("p (d c) -> p d c", c=PS_COLS)
            pd3 = ps_d[:].rearrange("(d p) c -> p d c", p=128)
            for d in range(4):
                nc.scalar.dma_start(ps3[:, d : d + 1, :], pd3[:, d : d + 1, :])
            nc.vector.memset(bias_sb[:, 5:6], 0.0)
            # touch the activation table during the DMA prologue so the
            # 1.3us LoadActFuncSet is off the critical path
            nc.scalar.activation(
                bias_sb[:, 5:6], bias_sb[:, 5:6],
                mybir.ActivationFunctionType.Relu,
            )
            wload(CCORDER[1])

            def conv(cc, side, poschunks=None):
                """Conv for channel chunk cc of one side -> cs/cq slab."""
                c0, ccw = CC0[cc], CCW[cc]
                src, dst, pdef, cols, slab, bcol = (
                    (ps_sb, cs_sb, POSCH_S, PS_COLS, SLAB_S, cc)
                    if side == 0
                    else (pq_sb, cq_sb, POSCH_Q, PQ_COLS, SLAB_Q, 5)
                )
                poschunks = poschunks or pdef
                src3 = src[:].rearrange("p (d c) -> p d c", c=cols)
                for pos0, pw in poschunks:
                    psum = convps.tile([128, 432], f32, tag="conv")
                    mms = []
                    for d in range(4):
                        # first and last matmul of the accumulation group must
                        # cover the full partition range (start/stop semantics
                        # are per-element), so full-size delta groups bracket
                        order = DORDER if d < 3 else [1, 0, 3, 4, 2]
                        for di in order:
                            delta, sz = DELTAS[di]
                            if sz <= c0:
                                continue
                            wcc = min(ccw, sz - c0)
                            mms.append((d, di, delta, wcc))
                    for idx, (d, di, delta, wcc) in enumerate(mms):
                        lcol = d * 2 * WSIDE + WCOL[(cc, side, di)]
                        rcol = pos0 + delta + 2
                        nc.tensor.matmul(
                            psum[0:wcc, 0:pw],
                            lhsT=w_sb[:, lcol : lcol + wcc],
                            rhs=src3[:, d, rcol : rcol + pw],
                            start=(idx == 0),
                            stop=(idx == len(mms) - 1),
                        )
                    # psum -> SBUF bf16 on the scalar engine; bias folds into
                    # the s side so each pairwise sum gets it exactly once
                    nc.scalar.add(
                        dst[0:ccw, cc * slab + pos0 : cc * slab + pos0 + pw],
                        psum[0:ccw, 0:pw],
                        bias_sb[0:ccw, bcol : bcol + 1],
                    )

            def pairwise(cc, last=False, first=False):
                """Fused pairwise add + maxpool for chunk cc.

                tmp[ch,j,i,l] = cs[ch,i,l] + cq[ch,j,l]   (DVE, 2x bf16)
                L1 16-wide max on DVE, L2..L5 on GPSIMD, relu on DVE.
                """
                ccw = CCW[cc]
                mx = mybir.AluOpType.max
                for j0, jb in JBLOCKS:
                    npr = jb * NROW
                    tmp = tmppool.tile([128, JN * NROW * 31], bf16, tag="tmp")
                    t16 = t16pool.tile([128, JN * NROW * 16], bf16, tag="t16")
                    t8 = t8pool.tile([128, JN * NROW * 8], bf16, tag="t8")
                    t4 = t4pool.tile([128, JN * NROW * 4], bf16, tag="t4")
                    t2 = t2pool.tile([128, JN * NROW * 2], bf16, tag="t2")
                    def ap(t, off, dims):
                        tap = t[:]
                        return bass.AP(
                            tap.tensor,
                            tap.offset + off,
                            [[tap.ap[0][0], ccw]] + dims,
                        )

                    # split by i only for the first chunk: lets the add
                    # start after the first s-poschunk lands (pipeline fill);
                    # elsewhere one instruction saves init overhead
                    isplit = ((0, 13), (13, 12)) if first else ((0, 25),)
                    for i0, ni in isplit:
                        nc.vector.tensor_tensor(
                            ap(tmp, i0 * 31, [[775, jb], [31, ni], [1, 31]]),
                            ap(cs_sb, cc * SLAB_S + 2 + i0 * ROWSTR,
                               [[0, jb], [ROWSTR, ni], [1, 31]]),
                            ap(cq_sb, cc * SLAB_Q + 2 + j0 * ROWSTR,
                               [[ROWSTR, jb], [0, ni], [1, 31]]),
                            op=mybir.AluOpType.add,
                        )
                    nc.vector.tensor_tensor(
                        ap(t16, 0, [[16, npr], [1, 16]]),
                        ap(tmp, 0, [[31, npr], [1, 16]]),
                        ap(tmp, 15, [[31, npr], [1, 16]]),
                        op=mx,
                    )
                    # tree levels L2..L5 + relu + store; for the last
                    # chunk run them in pair-range pieces so the final out
                    # DMA overlaps the tail of the tree
                    pieces = ((0, 225), (225, 100)) if last else ((0, npr),)
                    for p0, pn in pieces:
                        nc.vector.tensor_tensor(
                            ap(t8, p0 * 8, [[8, pn], [1, 8]]),
                            ap(t16, p0 * 16, [[16, pn], [1, 8]]),
                            ap(t16, p0 * 16 + 8, [[16, pn], [1, 8]]),
                            op=mx,
                        )
                        nc.vector.tensor_tensor(
                            ap(t4, p0 * 4, [[4, pn], [1, 4]]),
                            ap(t8, p0 * 8, [[8, pn], [1, 4]]),
                            ap(t8, p0 * 8 + 4, [[8, pn], [1, 4]]),
                            op=mx,
                        )
                        nc.vector.tensor_tensor(
                            ap(t2, p0 * 2, [[2, pn], [1, 2]]),
                            ap(t4, p0 * 4, [[4, pn], [1, 2]]),
                            ap(t4, p0 * 4 + 2, [[4, pn], [1, 2]]),
                            op=mx,
                        )
                        nc.vector.tensor_tensor(
                            ap(red, cc * NPAIR + j0 * NROW + p0, [[1, pn]]),
                            ap(t2, p0 * 2, [[2, pn]]),
                            ap(t2, p0 * 2 + 1, [[2, pn]]),
                            op=mx,
                        )
                        if last:
                            nc.vector.tensor_scalar_max(
                                red[0:ccw, cc * NPAIR + p0 : cc * NPAIR + p0 + pn],
                                red[0:ccw, cc * NPAIR + p0 : cc * NPAIR + p0 + pn],
                                0.0,
                            )
                            nc.sync.dma_start(
                                out_d[CC0[cc] : CC0[cc] + ccw, p0 : p0 + pn],
                                red[0:ccw, cc * NPAIR + p0 : cc * NPAIR + p0 + pn],
                            )
                if not last:
                    nc.scalar.activation(
                        red[0:ccw, cc * NPAIR : (cc + 1) * NPAIR],
                        red[0:ccw, cc * NPAIR : (cc + 1) * NPAIR],
                        mybir.ActivationFunctionType.Relu,
                    )
                    nc.sync.dma_start(
                        out_d[CC0[cc] : CC0[cc] + ccw, :],
                        red[0:ccw, cc * NPAIR : (cc + 1) * NPAIR],
                    )

            # software pipeline: conv leads pairwise by one chunk
            conv(CCORDER[0], 1)
            conv(CCORDER[0], 0)
            for k in range(1, 5):
                if k + 1 <= 4:
                    wload(CCORDER[k + 1])
                conv(CCORDER[k], 1)
                conv(CCORDER[k], 0)
                pairwise(CCORDER[k - 1], first=(k == 1))
            pairwise(CCORDER[4], last=True)

    nc.compile()
    return nc


def get_program():
    global _PROG
    if _PROG is None:
        _PROG = _build_program()
    return _PROG


def build_inputs(s, q, ws, bs):
    """Host-side shard prep. ws/bs: dicts k -> w(150, 1024, k) / b(150,).

    Returns in_maps. Core c handles episode c//2, q-row half c%2.
    """
    s = np.asarray(s, dtype=np.float32).reshape(B, NROW, L, D)
    q = np.asarray(q, dtype=np.float32).reshape(B, NQROW, L, D)

    # packed weights [D, 2*2100]: per side, delta groups at WOFF offsets,
    # device channel order [k5|k4|k3|k2]
    wall = np.zeros((D, 2 * WSIDE), dtype=np.float32)
    bias_dev = np.zeros(640, dtype=np.float32)
    for k in (2, 3, 4, 5):
        blk = ORD_OF_K[k] * 150
        bias_dev[blk : blk + 150] = bs[k]
        for di, (delta, sz) in enumerate(DELTAS):
            t = delta + PAD_OF_K[k]
            if not (0 <= t < k):
                continue
            assert blk + 150 <= sz
            wall[:, WOFF[di] + blk : WOFF[di] + blk + 150] = ws[k][:, :D, t].T
            wall[:, WSIDE + WOFF[di] + blk : WSIDE + WOFF[di] + blk + 150] = (
                ws[k][:, D:, t].T
            )
    perm = np.zeros(2 * WSIDE, dtype=np.int64)
    for side in range(2):
        for di, (_, sz) in enumerate(DELTAS):
            for cc in range(5):
                c0 = CC0[cc]
                if sz <= c0:
                    continue
                w = min(128, sz - c0)
                newc = WCOL[(cc, side, di)]
                oldc = side * WSIDE + WOFF[di] + c0
                perm[newc : newc + w] = np.arange(oldc, oldc + w)
    wall = wall[:, perm].astype(BF16)
    bias_col = bias_dev[:, None]

    in_maps = []
    for core in range(8):
        b, jh = core // 2, core % 2
        jidx = [min(jh * JN + t, NQROW - 1) for t in range(JN)]
        psa = np.zeros((D, PS_COLS), dtype=np.float32)
        pqa = np.zeros((D, PQ_COLS), dtype=np.float32)
        for r in range(NROW):
            psa[:, r * ROWSTR + 4 : r * ROWSTR + 4 + L] = s[b, r].T
        for t, j in enumerate(jidx):
            pqa[:, t * ROWSTR + 4 : t * ROWSTR + 4 + L] = q[b, j].T
        in_maps.append(
            {
                "ps": psa.astype(BF16),
                "pq": pqa.astype(BF16),
                "w": wall,
                "bias": bias_col,
            }
        )
    return in_maps


# device channel -> original output channel maps
_S_IDX = np.array(
    [(3 - g) * 150 + u for g in range(4) for u in range(75)], dtype=np.int64
)
_Q_IDX = _S_IDX + 75


def assemble_outputs(core_outs):
    """core_outs: list of 8 arrays [NCH, NPAIR] -> (s_out, q_out)."""
    s_out = np.empty((B, NROW, NQROW, 300), dtype=np.float32)
    q_out = np.empty((B, NROW, NQROW, 300), dtype=np.float32)
    for core in range(8):
        b, jh = core // 2, core % 2
        nj = JN if jh == 0 else NQROW - JN
        # out[ch, j*25+i] -> [j, i, ch]
        arr = (
            np.asarray(core_outs[core])
            .astype(np.float32)
            .reshape(NCH, JN, NROW)
            .transpose(1, 2, 0)
        )
        s_out[b, :, jh * JN : jh * JN + nj] = arr[:nj][:, :, _S_IDX].transpose(
            1, 0, 2
        )
        q_out[b, :, jh * JN : jh * JN + nj] = arr[:nj][:, :, _Q_IDX].transpose(
            1, 0, 2
        )
    return s_out.reshape(-1, 300), q_out.reshape(-1, 300)


def kernel(s, q, w2, b2, w3, b3, w4, b4, w5, b5, B=4, N=5, K=5, Q=5, L=31):
    ws = {2: np.asarray(w2, np.float32), 3: np.asarray(w3, np.float32),
          4: np.asarray(w4, np.float32), 5: np.asarray(w5, np.float32)}
    bs = {2: np.asarray(b2, np.float32), 3: np.asarray(b3, np.float32),
          4: np.asarray(b4, np.float32), 5: np.asarray(b5, np.float32)}
    in_maps = build_inputs(s, q, ws, bs)
    nc = get_program()
    res = run_bass_kernel_spmd(nc, in_maps, list(range(8))).results
    return assemble_outputs([res[c]["out"] for c in range(8)])
